# revision 47
# baseline (speedup 1.0000x reference)
"""Trainium2 Bass kernel for BertSelfAttention with relative_key_query position bias.

Full inputs in, full output out; internally sharded over 8 NeuronCores:
data-parallel over batch (B=2) x tensor-parallel over heads (16 heads -> 4 per core).

Per-core algorithm (b fixed, 4 heads, S=2048, d=64), all-fp16 matmul inputs:
  qT,kT = Wq/Wk_slice.T @ x.T   (d-on-partition layout), v natural [S, d]
  scoresT[r,l] = k[r].q[l] + U[l, l-r+2047] + V[r, l-r+2047]
      U = q @ E^T, V = k @ E^T  (E = dist_emb), computed as diagonal BANDS on PE,
      staged to DRAM (sheared for U / plain for V) so the diagonal extraction is
      a plain strided DMA (V) and a 2-byte xbar transpose DMA (U).
  Mega-phase structure: ALL heads' bands first, then all assembly, so the
  diagonal readback DMAs prefetch deeply and the PE never stalls (keeps the
  tensor engine p-state at full clock).
  exp via ScalarE (scale=1/8), unnormalized; softmax sums come free from a
  ones-augmented v in the PV matmul (row 64 of ctxT = sum_r exp).
  V-injection alternates DVE tensor_add / PE identity-matmul to balance engines.
  ctx = (exp^T @ v-hat) / sums, transposed back on PE, written out.
"""

import sys

for _p in ("/opt/trn_rl_repo", "/root/.axon_site/_ro/trn_rl_repo"):
    if _p not in sys.path:
        sys.path.insert(0, _p)

import numpy as np

import concourse.bass as bass
import concourse.tile as tile
from concourse import bacc, mybir
from concourse.bass_utils import run_bass_kernel_spmd
from concourse.masks import make_identity

F32 = mybir.dt.float32
F16 = mybir.dt.float16

N_CORES = 8


def build_program(S=2048, HIN=1024, NH=4, D=64):
    DC = NH * D            # per-core projection output cols
    KC = HIN // 128        # contraction chunks
    NLB = S // 128         # l-blocks / r-tiles
    NLC = 2                # big l-chunks for assembly
    LCW = S // NLC         # l-chunk width
    SH = S - 1             # distance shift (= MAX_POS-1)
    BW = S + 128           # band width per 128-row block (written span)
    BP = BW                # V band row pitch
    BPU = S + 256          # U band row pitch (sheared row span is 2303)
    EJ = 2 * S             # padded E^T cols (E rows 2S-1, pad 1)
    NPAIR = NH // 2

    nc = bacc.Bacc(None, target_bir_lowering=False, debug=False, num_devices=N_CORES)

    xt = nc.declare_dram_parameter("xt", [KC, 128, S], F16, isOutput=False)
    wq = nc.declare_dram_parameter("wq", [KC, 128, DC], F16, isOutput=False)
    wk = nc.declare_dram_parameter("wk", [KC, 128, DC], F16, isOutput=False)
    wv = nc.declare_dram_parameter("wv", [KC, 128, DC], F16, isOutput=False)
    bq = nc.declare_dram_parameter("bq", [DC], F32, isOutput=False)
    bk = nc.declare_dram_parameter("bk", [DC], F32, isOutput=False)
    bv = nc.declare_dram_parameter("bv", [DC], F32, isOutput=False)
    et = nc.declare_dram_parameter("et", [128, 2 * EJ], F16, isOutput=False)
    out = nc.declare_dram_parameter("out", [S, DC], F32, isOutput=True)

    with tile.TileContext(nc) as tc:
        _body(tc, locals())
    nc.compile()
    return nc


def _body(tc, g):
    nc = tc.nc
    S, NH, D, KC, DC = g["S"], g["NH"], g["D"], g["KC"], g["DC"]
    NLB, NLC, LCW, SH, BW, BP, BPU, EJ, NPAIR = (
        g["NLB"], g["NLC"], g["LCW"], g["SH"], g["BW"], g["BP"], g["BPU"],
        g["EJ"], g["NPAIR"])
    xt, wq, wk, wv, bq, bk, bv, et, out = (
        g["xt"], g["wq"], g["wk"], g["wv"], g["bq"], g["bk"], g["bv"], g["et"], g["out"])

    ID = mybir.ActivationFunctionType.Identity
    EXP = mybir.ActivationFunctionType.Exp

    # band chunking: [0,1024) + [1024,2048) psum tiles, [2048,2176) tail tile
    BCHA = [(0, 512), (512, 512)]
    BCHB = [(1024, 512), (1536, 512)]

    from contextlib import ExitStack
    ctx = ExitStack()
    with ctx:
        constp = ctx.enter_context(tc.tile_pool(name="const", bufs=1))
        qkvp = ctx.enter_context(tc.tile_pool(name="qkv", bufs=1))
        ctxp = ctx.enter_context(tc.tile_pool(name="ctxsb", bufs=NH))
        dramp = ctx.enter_context(tc.tile_pool(name="dram", bufs=1, space="DRAM"))

        et_sb = constp.tile([128, 2 * EJ], F16)
        id16 = constp.tile([128, 128], F16)
        make_identity(nc, id16)
        id32 = constp.tile([128, 128], F32)
        make_identity(nc, id32)
        bq_sb = constp.tile([128, NPAIR], F32)
        bk_sb = constp.tile([128, NPAIR], F32)
        for dc in range(NPAIR):
            nc.sync.dma_start(out=bq_sb[:, dc:dc + 1],
                              in_=bass.AP(tensor=bq, offset=dc * 128,
                                          ap=[[1, 128], [0, 1]]))
            nc.sync.dma_start(out=bk_sb[:, dc:dc + 1],
                              in_=bass.AP(tensor=bk, offset=dc * 128,
                                          ap=[[1, 128], [0, 1]]))
        bvb = constp.tile([128, DC], F32)
        nc.gpsimd.dma_start(out=bvb, in_=bass.AP(tensor=bv, offset=0,
                                                 ap=[[0, 128], [1, DC]]))

        qt = [qkvp.tile([128, S], F16, name=f"qt{p}", tag=f"qt{p}") for p in range(NPAIR)]
        kt = [qkvp.tile([128, S], F16, name=f"kt{p}", tag=f"kt{p}") for p in range(NPAIR)]
        # 63 tail cols pad the last head's PV stationary slice to M=128
        vhat = qkvp.tile([128, NLB, NH * 65 + 63], F16)
        ctxT = [ctxp.tile([65, S], F32, name=f"ctxT{h}", tag="ctxT") for h in range(NH)]

        # ---------------- Phase 1: projections ----------------
        with tc.tile_pool(name="xtw", bufs=1) as xtwp, \
             tc.tile_pool(name="pps", bufs=2, space="PSUM") as pps:
            xt_sb = xtwp.tile([128, KC, S], F16)
            nc.sync.dma_start(
                out=xt_sb,
                in_=bass.AP(tensor=xt, offset=0,
                            ap=[[S, 128], [128 * S, KC], [1, S]]))
            w_sb = {}
            for nm, wt in (("q", wq), ("k", wk), ("v", wv)):
                w_sb[nm] = xtwp.tile([128, KC, DC], F16, name=f"w{nm}", tag=f"w{nm}")
                nc.gpsimd.dma_start(
                    out=w_sb[nm],
                    in_=bass.AP(tensor=wt, offset=0,
                                ap=[[DC, 128], [128 * DC, KC], [1, DC]]))
            # et is only needed by the band phase; load it after the weights
            nc.gpsimd.dma_start(out=et_sb, in_=et[:, :])

            PSW = 512
            for dc in range(NPAIR):
                for sc in range(S // PSW):
                    psq = pps.tile([128, PSW], F32, tag="psq")
                    psk = pps.tile([128, PSW], F32, tag="psk")
                    for k8 in range(KC):
                        nc.tensor.matmul(psq, w_sb["q"][:, k8, dc * 128:dc * 128 + 128],
                                         xt_sb[:, k8, sc * PSW:sc * PSW + PSW],
                                         start=(k8 == 0), stop=(k8 == KC - 1))
                    for k8 in range(KC):
                        nc.tensor.matmul(psk, w_sb["k"][:, k8, dc * 128:dc * 128 + 128],
                                         xt_sb[:, k8, sc * PSW:sc * PSW + PSW],
                                         start=(k8 == 0), stop=(k8 == KC - 1))
                    nc.scalar.activation(out=qt[dc][:, sc * PSW:sc * PSW + PSW], in_=psq,
                                         func=ID, bias=bq_sb[:, dc:dc + 1], scale=1.0)
                    nc.scalar.activation(out=kt[dc][:, sc * PSW:sc * PSW + PSW], in_=psk,
                                         func=ID, bias=bk_sb[:, dc:dc + 1], scale=1.0)

            for st in range(NLB):
                psv = pps.tile([128, DC], F32, tag="psv")
                for k8 in range(KC):
                    nc.tensor.matmul(psv, xt_sb[:, k8, st * 128:st * 128 + 128],
                                     w_sb["v"][:, k8, :],
                                     start=(k8 == 0), stop=(k8 == KC - 1))
                nc.vector.tensor_copy(
                    out=vhat[:, st, 0:NH * 65].rearrange("p (h c) -> p h c", h=NH)[:, :, 0:64],
                    in_=psv.rearrange("p (h c) -> p h c", h=NH))
            nc.vector.memset(vhat[:, :, 0:NH * 65].rearrange("p s (h c) -> p s h c", h=NH)[:, :, :, 64:65], 1.0)
            nc.vector.memset(vhat[:, :, NH * 65:], 0.0)

        # per-PAIR bands: each fp16 cell packs (fp8 h_even, fp8 h_odd)
        ub = [dramp.tile([NLB * 128, BPU], F16, name=f"ub{p}", tag=f"ub{p}")
              for p in range(NPAIR)]
        vb = [dramp.tile([NLB * 128, BP], F16, name=f"vb{p}", tag=f"vb{p}")
              for p in range(NPAIR)]

        # ---------------- Phase 2: interleaved bands + assembly ----------------
        # One unified PSUM ring (4 banks) serves both band chunks and score
        # tiles; one [65, S] PV accumulator per head (4 banks).  The SP HWDGE
        # queue carries ONLY the xbar transposes (concurrent transposes on two
        # HWDGE queues corrupt each other through the shared xbar, measured),
        # so it streams them continuously; band writes and v_d reads ride the
        # gpsimd SWDGE queue.  Head h's assembly is interleaved with head
        # h+1's band creation to keep every engine and the xbar busy.
        st_load = {"act": 0.0, "dve": 0.0}

        def stage(dst, src, cols):
            ca = st_load["act"] + cols * 0.833 + 280
            cd = st_load["dve"] + cols * 1.042 + 300
            if ca <= cd:
                st_load["act"] = ca
                nc.scalar.copy(out=dst, in_=src)
            else:
                st_load["dve"] = cd
                nc.vector.tensor_copy(out=dst, in_=src)

        SCW = 512
        F8 = mybir.dt.float8e4
        id8 = constp.tile([128, 128], F8)
        nc.vector.tensor_copy(out=id8, in_=id32)

        with tc.tile_pool(name="stg", bufs=2) as stgp, \
             tc.tile_pool(name="udp", bufs=9) as udp, \
             tc.tile_pool(name="vdp", bufs=4) as vdp, \
             tc.tile_pool(name="expp", bufs=22) as expp:

            def band_block(bpool, src, st8, d, hh, jref, psl, tp):
                # one 128-row band block: cols [0, BW) of moving E at jref,
                # staged into fp8 lane hh of the packed fp16 cells
                for (c0, cw) in ((0, 1024), (1024, 1024), (2048, 128)):
                    ps = bpool.tile([128, 1024], F32, tag="ps")
                    for s2 in range(0, cw, SCW):
                        w = min(SCW, cw - s2)
                        nc.tensor.matmul(ps[:, s2:s2 + w], src,
                                         et_sb[psl, jref + c0 + s2:jref + c0 + s2 + w],
                                         start=True, stop=True, tile_position=tp)
                    stage(st8[:, d, c0:c0 + cw, hh], ps[:, 0:cw], cw)

            def band_group(bpool, pair, g2):
                l00 = g2 * 128
                # U band rows l, sheared via reversed E at cols [EJ, 2EJ)
                stg = stgp.tile([128, 2, BW], F16, tag="stg")
                st8 = stg.bitcast(F8).rearrange("p d (c two) -> p d c two", two=2)
                for d in range(2):
                    l0 = l00 + d * 128
                    for hh in range(2):
                        psl = slice(hh * 64, hh * 64 + 64)
                        band_block(bpool, qt[pair][psl, l0:l0 + 128], st8, d, hh,
                                   (S - 128) - l0 + EJ, psl, (hh * 64, 0))
                nc.gpsimd.dma_start(
                    out=bass.AP(tensor=ub[pair].tensor,
                                offset=ub[pair].offset + l00 * BPU,
                                ap=[[BPU + 1, 128], [128 * BPU, 2], [1, BW]]),
                    in_=stg)
                # V band rows r, plain: vb[r, j - jbt], jbt = S-128 - r0
                stgv = stgp.tile([128, 2, BW], F16, tag="stgv")
                sv8 = stgv.bitcast(F8).rearrange("p d (c two) -> p d c two", two=2)
                for d in range(2):
                    r0 = l00 + d * 128
                    for hh in range(2):
                        psl = slice(hh * 64, hh * 64 + 64)
                        band_block(bpool, kt[pair][psl, r0:r0 + 128], sv8, d, hh,
                                   (S - 128) - r0, psl, (hh * 64, 0))
                nc.gpsimd.dma_start(
                    out=bass.AP(tensor=vb[pair].tensor,
                                offset=vb[pair].offset + l00 * BP,
                                ap=[[BP, 128], [128 * BP, 2], [1, BW]]),
                    in_=stgv)

            def asm_pair(pair, psr, pvps):
                ebuf = {(hh, lc): [] for hh in range(2) for lc in range(NLC)}
                ready = []

                def do_pv(item):
                    hh, lc, pend, first = item
                    head = 2 * pair + hh
                    # M=128 stationary (65-col stationaries run the PE at half
                    # rate); rows 65:128 of pvt are junk
                    vh = slice(head * 65, head * 65 + 128)
                    lco = lc * LCW
                    pvt = pvps.tile([128, LCW], F32, tag="pv")
                    for k, (rtk, e) in enumerate(pend):
                        for s2 in range(LCW // SCW):
                            s_sl = slice(s2 * SCW, s2 * SCW + SCW)
                            nc.tensor.matmul(pvt[:, s_sl], vhat[:, rtk, vh],
                                             e[:, s_sl], start=(k == 0),
                                             stop=(k == len(pend) - 1))
                    csl = ctxT[head][:, lco:lco + LCW]
                    if first:
                        nc.vector.tensor_copy(out=csl, in_=pvt[0:65, :])
                    else:
                        nc.vector.tensor_add(csl, csl, pvt[0:65, :])

                for rt in range(NLB):
                    r0 = rt * 128
                    # one packed full-row transpose readback per r-tile
                    u_d = udp.tile([128, S], F16, tag="u_d")
                    nc.sync.dma_start(
                        out=u_d,
                        in_=bass.AP(tensor=ub[pair].tensor,
                                    offset=ub[pair].offset + 128 + r0,
                                    ap=[[BPU, S], [1, 128]]),
                        transpose=True)
                    u8 = u_d.bitcast(F8).rearrange("p (c two) -> p c two", two=2)
                    v_d = vdp.tile([128, S], F16, tag="v_d")
                    nc.sync.dma_start(
                        out=v_d,
                        in_=bass.AP(tensor=vb[pair].tensor,
                                    offset=vb[pair].offset + r0 * BP + 127,
                                    ap=[[BP - 1, 128], [1, S]]))
                    v8 = v_d.bitcast(F8).rearrange("p (c two) -> p c two", two=2)
                    for hh in range(2):
                        psl = slice(hh * 64, hh * 64 + 64)
                        tp = (hh * 64, 0)
                        for lc in range(NLC):
                            lco = lc * LCW
                            sc = psr.tile([128, LCW], F32, tag="ps")
                            for s2 in range(LCW // SCW):
                                s_sl = slice(s2 * SCW, s2 * SCW + SCW)
                                nc.tensor.matmul(
                                    sc[:, s_sl], kt[pair][psl, r0:r0 + 128],
                                    qt[pair][psl, lco + s2 * SCW:lco + s2 * SCW + SCW],
                                    start=True, stop=False, tile_position=tp)
                            for s2 in range(LCW // SCW):
                                s_sl = slice(s2 * SCW, s2 * SCW + SCW)
                                nc.tensor.matmul(
                                    sc[:, s_sl], id8,
                                    u8[:, lco + s2 * SCW:lco + s2 * SCW + SCW, hh],
                                    start=False, stop=False)
                            for s2 in range(LCW // SCW):
                                s_sl = slice(s2 * SCW, s2 * SCW + SCW)
                                nc.tensor.matmul(
                                    sc[:, s_sl], id8,
                                    v8[:, lco + s2 * SCW:lco + s2 * SCW + SCW, hh],
                                    start=False, stop=True)
                            e_t = expp.tile([128, LCW], F16, tag="e_t")
                            nc.scalar.activation(out=e_t, in_=sc, func=EXP,
                                                 scale=1.0 / np.sqrt(D))
                            ebuf[(hh, lc)].append((rt, e_t))
                    if rt % 2 == 1:
                        # enqueue the closed 2-rt PV window per stream; process
                        # with one-window delay so the trailing exps are never
                        # on the PE's critical path
                        for hh in range(2):
                            for lc in range(NLC):
                                ready.append((hh, lc, ebuf[(hh, lc)], rt == 1))
                                ebuf[(hh, lc)] = []
                        while len(ready) > 4:
                            do_pv(ready.pop(0))
                while ready:
                    do_pv(ready.pop(0))

            # both band phases first (deep dedicated PSUM ring, 8 banks), so
            # the pair-0 xbar readbacks prefetch during the pair-1 band phase
            with tc.tile_pool(name="psrB", bufs=4, space="PSUM") as psrB:
                for pair in range(NPAIR):
                    for g2 in range(0, NLB, 2):
                        band_group(psrB, pair, g2)
            # then both assembly phases: clean PE stream at full clock
            with tc.tile_pool(name="psr", bufs=2, space="PSUM") as psr, \
                 tc.tile_pool(name="pvps", bufs=2, space="PSUM") as pvps:
                for pair in range(NPAIR):
                    asm_pair(pair, psr, pvps)

        # ---------------- Phase 3: transpose ctx, normalize, write out ----------------
        with tc.tile_pool(name="fps", bufs=2, space="PSUM") as fps, \
             tc.tile_pool(name="fsb", bufs=3) as fsb:
            for lt in range(NLB):
                ot = fsb.tile([128, DC], F32, tag="ot")
                for head in range(NH):
                    tp_ps = fps.tile([128, 128], F32, tag="tp_ps")
                    nc.tensor.matmul(tp_ps[:, 0:65], ctxT[head][:, lt * 128:lt * 128 + 128],
                                     id32[0:65, 0:65], is_transpose=True)
                    rec = fsb.tile([128, 1], F32, tag="rec")
                    nc.vector.reciprocal(rec, tp_ps[:, 64:65])
                    nc.vector.tensor_scalar_mul(ot[:, head * 64:head * 64 + 64],
                                                tp_ps[:, 0:64], rec)
                nc.vector.tensor_add(ot, ot, bvb)
                nc.gpsimd.dma_start(out=out[lt * 128:lt * 128 + 128, :], in_=ot)


_PROG = {}


def _get_prog():
    if "p" not in _PROG:
        _PROG["p"] = build_program()
    return _PROG["p"]


def make_in_maps(hidden_states, Wq, bq, Wk, bk, Wv, bv, dist_emb):
    S, HIN = 2048, 1024
    hidden_states = np.asarray(hidden_states, dtype=np.float32)
    Wq = np.asarray(Wq, dtype=np.float16)
    Wk = np.asarray(Wk, dtype=np.float16)
    Wv = np.asarray(Wv, dtype=np.float16)
    bq = np.asarray(bq, dtype=np.float32)
    bk = np.asarray(bk, dtype=np.float32)
    bv = np.asarray(bv, dtype=np.float32)
    dist_emb = np.asarray(dist_emb, dtype=np.float16)

    etp = np.zeros((64, 2 * S), np.float16)
    etp[:, : 2 * S - 1] = dist_emb.T
    # reversed copy for the U band (forward-stride writes): E'[j'] = E[2S-1-j']
    etr = np.zeros((64, 2 * S), np.float16)
    etr[:, 1:] = dist_emb.T[:, ::-1]
    eth = np.concatenate([etp, etr], axis=1)
    et_full = np.ascontiguousarray(np.concatenate([eth, eth], axis=0))

    in_maps = []
    for c in range(N_CORES):
        b = c // 4
        q = c % 4
        cols = slice(q * 256, q * 256 + 256)
        xt = np.ascontiguousarray(hidden_states[b].T.astype(np.float16)).reshape(
            HIN // 128, 128, S)
        in_maps.append({
            "xt": xt,
            "wq": np.ascontiguousarray(Wq[:, cols]).reshape(HIN // 128, 128, 256),
            "wk": np.ascontiguousarray(Wk[:, cols]).reshape(HIN // 128, 128, 256),
            "wv": np.ascontiguousarray(Wv[:, cols]).reshape(HIN // 128, 128, 256),
            "bq": np.ascontiguousarray(bq[cols]),
            "bk": np.ascontiguousarray(bk[cols]),
            "bv": np.ascontiguousarray(bv[cols]),
            "et": et_full,
        })
    return in_maps


def assemble_output(results, B=2, S=2048, HIN=1024):
    full = np.empty((B, S, HIN), np.float32)
    for c in range(N_CORES):
        b = c // 4
        q = c % 4
        full[b, :, q * 256:q * 256 + 256] = results[c]["out"]
    return full


def kernel(hidden_states, Wq, bq, Wk, bk, Wv, bv, dist_emb):
    nc = _get_prog()
    in_maps = make_in_maps(hidden_states, Wq, bq, Wk, bk, Wv, bv, dist_emb)
    res = run_bass_kernel_spmd(nc, in_maps, list(range(N_CORES)))
    return assemble_output(res.results, B=np.asarray(hidden_states).shape[0])


if __name__ == "__main__":
    rng = np.random.default_rng(0)
    B, S, H = 2, 2048, 1024
    inputs = {
        "hidden_states": rng.standard_normal((B, S, H), dtype=np.float32),
        "Wq": rng.standard_normal((H, H), dtype=np.float32) / 32,
        "bq": np.zeros(H, np.float32),
        "Wk": rng.standard_normal((H, H), dtype=np.float32) / 32,
        "bk": np.zeros(H, np.float32),
        "Wv": rng.standard_normal((H, H), dtype=np.float32) / 32,
        "bv": np.zeros(H, np.float32),
        "dist_emb": (rng.standard_normal((2 * 2048 - 1, 64)) * 0.02).astype(np.float32),
    }
    out = kernel(**inputs)
    print("kernel output", out.shape, out.dtype)


# revision 48
# speedup vs baseline: 1.1820x; 1.1820x over previous
"""Trainium2 Bass kernel for BertSelfAttention with relative_key_query position bias.

Full inputs in, full output out; internally sharded over 8 NeuronCores:
data-parallel over batch (B=2) x tensor-parallel over heads (16 heads -> 4 per core).

Per-core algorithm (b fixed, 4 heads, S=2048, d=64), all-fp16 matmul inputs:
  qT,kT = Wq/Wk_slice.T @ x.T   (d-on-partition layout), v natural [S, d]
  scoresT[r,l] = k[r].q[l] + U[l, l-r+2047] + V[r, l-r+2047]
      U = q @ E^T, V = k @ E^T  (E = dist_emb), computed as diagonal BANDS on PE,
      staged to DRAM (sheared for U / plain for V) so the diagonal extraction is
      a plain strided DMA (V) and a 2-byte xbar transpose DMA (U).
  Mega-phase structure: ALL heads' bands first, then all assembly, so the
  diagonal readback DMAs prefetch deeply and the PE never stalls (keeps the
  tensor engine p-state at full clock).
  exp via ScalarE (scale=1/8), unnormalized; softmax sums come free from a
  ones-augmented v in the PV matmul (row 64 of ctxT = sum_r exp).
  V-injection alternates DVE tensor_add / PE identity-matmul to balance engines.
  ctx = (exp^T @ v-hat) / sums, transposed back on PE, written out.
"""

import sys

for _p in ("/opt/trn_rl_repo", "/root/.axon_site/_ro/trn_rl_repo"):
    if _p not in sys.path:
        sys.path.insert(0, _p)

import numpy as np

import concourse.bass as bass
import concourse.tile as tile
from concourse import bacc, mybir
from concourse.bass_utils import run_bass_kernel_spmd
from concourse.masks import make_identity

F32 = mybir.dt.float32
F16 = mybir.dt.float16

N_CORES = 8


def build_program(S=2048, HIN=1024, NH=4, D=64):
    DC = NH * D            # per-core projection output cols
    KC = HIN // 128        # contraction chunks
    NLB = S // 128         # l-blocks / r-tiles
    NLC = 2                # big l-chunks for assembly
    LCW = S // NLC         # l-chunk width
    SH = S - 1             # distance shift (= MAX_POS-1)
    BW = S + 128           # band width per 128-row block (written span)
    BP = BW                # V band row pitch
    BPU = S + 256          # U band row pitch (sheared row span is 2303)
    EJ = 2 * S             # padded E^T cols (E rows 2S-1, pad 1)
    NPAIR = NH // 2

    nc = bacc.Bacc(None, target_bir_lowering=False, debug=False, num_devices=N_CORES)

    xt = nc.declare_dram_parameter("xt", [KC, 128, S], F16, isOutput=False)
    wq = nc.declare_dram_parameter("wq", [KC, 128, DC], F16, isOutput=False)
    wk = nc.declare_dram_parameter("wk", [KC, 128, DC], F16, isOutput=False)
    wv = nc.declare_dram_parameter("wv", [KC, 128, DC], F16, isOutput=False)
    bq = nc.declare_dram_parameter("bq", [DC], F32, isOutput=False)
    bk = nc.declare_dram_parameter("bk", [DC], F32, isOutput=False)
    bv = nc.declare_dram_parameter("bv", [DC], F32, isOutput=False)
    et = nc.declare_dram_parameter("et", [128, 2 * EJ], F16, isOutput=False)
    out = nc.declare_dram_parameter("out", [S, DC], F32, isOutput=True)

    with tile.TileContext(nc) as tc:
        _body(tc, locals())
    nc.compile()
    return nc


def _body(tc, g):
    nc = tc.nc
    S, NH, D, KC, DC = g["S"], g["NH"], g["D"], g["KC"], g["DC"]
    NLB, NLC, LCW, SH, BW, BP, BPU, EJ, NPAIR = (
        g["NLB"], g["NLC"], g["LCW"], g["SH"], g["BW"], g["BP"], g["BPU"],
        g["EJ"], g["NPAIR"])
    xt, wq, wk, wv, bq, bk, bv, et, out = (
        g["xt"], g["wq"], g["wk"], g["wv"], g["bq"], g["bk"], g["bv"], g["et"], g["out"])

    ID = mybir.ActivationFunctionType.Identity
    EXP = mybir.ActivationFunctionType.Exp

    # band chunking: [0,1024) + [1024,2048) psum tiles, [2048,2176) tail tile
    BCHA = [(0, 512), (512, 512)]
    BCHB = [(1024, 512), (1536, 512)]

    from contextlib import ExitStack
    ctx = ExitStack()
    with ctx:
        constp = ctx.enter_context(tc.tile_pool(name="const", bufs=1))
        qkvp = ctx.enter_context(tc.tile_pool(name="qkv", bufs=1))
        ctxp = ctx.enter_context(tc.tile_pool(name="ctxsb", bufs=NH))
        dramp = ctx.enter_context(tc.tile_pool(name="dram", bufs=1, space="DRAM"))

        et_sb = constp.tile([128, 2 * EJ], F16)
        id16 = constp.tile([128, 128], F16)
        make_identity(nc, id16)
        id32 = constp.tile([128, 128], F32)
        make_identity(nc, id32)
        bq_sb = constp.tile([128, NPAIR], F32)
        bk_sb = constp.tile([128, NPAIR], F32)
        for dc in range(NPAIR):
            nc.sync.dma_start(out=bq_sb[:, dc:dc + 1],
                              in_=bass.AP(tensor=bq, offset=dc * 128,
                                          ap=[[1, 128], [0, 1]]))
            nc.sync.dma_start(out=bk_sb[:, dc:dc + 1],
                              in_=bass.AP(tensor=bk, offset=dc * 128,
                                          ap=[[1, 128], [0, 1]]))
        bvb = constp.tile([128, DC], F32)
        nc.gpsimd.dma_start(out=bvb, in_=bass.AP(tensor=bv, offset=0,
                                                 ap=[[0, 128], [1, DC]]))

        qt = [qkvp.tile([128, S], F16, name=f"qt{p}", tag=f"qt{p}") for p in range(NPAIR)]
        kt = [qkvp.tile([128, S], F16, name=f"kt{p}", tag=f"kt{p}") for p in range(NPAIR)]
        # 63 tail cols pad the last head's PV stationary slice to M=128
        vhat = qkvp.tile([128, NLB, NH * 65 + 63], F16)
        ctxT = [ctxp.tile([65, S], F32, name=f"ctxT{h}", tag="ctxT") for h in range(NH)]

        # ---------------- Phase 1: projections ----------------
        with tc.tile_pool(name="xtw", bufs=1) as xtwp, \
             tc.tile_pool(name="pps", bufs=2, space="PSUM") as pps:
            xt_sb = xtwp.tile([128, KC, S], F16)
            nc.sync.dma_start(
                out=xt_sb,
                in_=bass.AP(tensor=xt, offset=0,
                            ap=[[S, 128], [128 * S, KC], [1, S]]))
            w_sb = {}
            for nm, wt in (("q", wq), ("k", wk), ("v", wv)):
                w_sb[nm] = xtwp.tile([128, KC, DC], F16, name=f"w{nm}", tag=f"w{nm}")
                nc.gpsimd.dma_start(
                    out=w_sb[nm],
                    in_=bass.AP(tensor=wt, offset=0,
                                ap=[[DC, 128], [128 * DC, KC], [1, DC]]))
            # et is only needed by the band phase; load it after the weights
            nc.gpsimd.dma_start(out=et_sb, in_=et[:, :])

            PSW = 512
            for dc in range(NPAIR):
                for sc in range(S // PSW):
                    psq = pps.tile([128, PSW], F32, tag="psq")
                    psk = pps.tile([128, PSW], F32, tag="psk")
                    for k8 in range(KC):
                        nc.tensor.matmul(psq, w_sb["q"][:, k8, dc * 128:dc * 128 + 128],
                                         xt_sb[:, k8, sc * PSW:sc * PSW + PSW],
                                         start=(k8 == 0), stop=(k8 == KC - 1))
                    for k8 in range(KC):
                        nc.tensor.matmul(psk, w_sb["k"][:, k8, dc * 128:dc * 128 + 128],
                                         xt_sb[:, k8, sc * PSW:sc * PSW + PSW],
                                         start=(k8 == 0), stop=(k8 == KC - 1))
                    nc.scalar.activation(out=qt[dc][:, sc * PSW:sc * PSW + PSW], in_=psq,
                                         func=ID, bias=bq_sb[:, dc:dc + 1], scale=1.0)
                    nc.scalar.activation(out=kt[dc][:, sc * PSW:sc * PSW + PSW], in_=psk,
                                         func=ID, bias=bk_sb[:, dc:dc + 1], scale=1.0)

            for st in range(NLB):
                psv = pps.tile([128, DC], F32, tag="psv")
                for k8 in range(KC):
                    nc.tensor.matmul(psv, xt_sb[:, k8, st * 128:st * 128 + 128],
                                     w_sb["v"][:, k8, :],
                                     start=(k8 == 0), stop=(k8 == KC - 1))
                nc.vector.tensor_copy(
                    out=vhat[:, st, 0:NH * 65].rearrange("p (h c) -> p h c", h=NH)[:, :, 0:64],
                    in_=psv.rearrange("p (h c) -> p h c", h=NH))
            nc.vector.memset(vhat[:, :, 0:NH * 65].rearrange("p s (h c) -> p s h c", h=NH)[:, :, :, 64:65], 1.0)
            nc.vector.memset(vhat[:, :, NH * 65:], 0.0)

        # per-PAIR bands: each fp16 cell packs (fp8 h_even, fp8 h_odd)
        ub = [dramp.tile([NLB * 128, BPU], F16, name=f"ub{p}", tag=f"ub{p}")
              for p in range(NPAIR)]
        vb = [dramp.tile([NLB * 128, BP], F16, name=f"vb{p}", tag=f"vb{p}")
              for p in range(NPAIR)]

        # ---------------- Phase 2: interleaved bands + assembly ----------------
        # One unified PSUM ring (4 banks) serves both band chunks and score
        # tiles; one [65, S] PV accumulator per head (4 banks).  The SP HWDGE
        # queue carries ONLY the xbar transposes (concurrent transposes on two
        # HWDGE queues corrupt each other through the shared xbar, measured),
        # so it streams them continuously; band writes and v_d reads ride the
        # gpsimd SWDGE queue.  Head h's assembly is interleaved with head
        # h+1's band creation to keep every engine and the xbar busy.
        st_load = {"act": 0.0, "dve": 0.0}

        def stage(dst, src, cols):
            ca = st_load["act"] + cols * 0.833 + 280
            cd = st_load["dve"] + cols * 1.042 + 300
            if ca <= cd:
                st_load["act"] = ca
                nc.scalar.copy(out=dst, in_=src)
            else:
                st_load["dve"] = cd
                nc.vector.tensor_copy(out=dst, in_=src)

        SCW = 512
        F8 = mybir.dt.float8e4
        id8 = constp.tile([128, 128], F8)
        nc.vector.tensor_copy(out=id8, in_=id32)

        with tc.tile_pool(name="stg", bufs=2) as stgp, \
             tc.tile_pool(name="udp", bufs=8) as udp, \
             tc.tile_pool(name="vdp", bufs=3) as vdp, \
             tc.tile_pool(name="w8p", bufs=2) as w8p, \
             tc.tile_pool(name="expp", bufs=21) as expp:

            def band_block(bpool, src, st8, d, hh, jref, psl, tp):
                # one 128-row band block: cols [0, BW) of moving E at jref,
                # staged into fp8 lane hh of the packed fp16 cells
                for (c0, cw) in ((0, 1024), (1024, 1024), (2048, 128)):
                    ps = bpool.tile([128, 1024], F32, tag="ps")
                    for s2 in range(0, cw, SCW):
                        w = min(SCW, cw - s2)
                        nc.tensor.matmul(ps[:, s2:s2 + w], src,
                                         et_sb[psl, jref + c0 + s2:jref + c0 + s2 + w],
                                         start=True, stop=True, tile_position=tp)
                    stage(st8[:, d, c0:c0 + cw, hh], ps[:, 0:cw], cw)

            def band_group(bpool, pair, g2):
                l00 = g2 * 128
                # U band rows l, sheared via reversed E at cols [EJ, 2EJ)
                stg = stgp.tile([128, 2, BW], F16, tag="stg")
                st8 = stg.bitcast(F8).rearrange("p d (c two) -> p d c two", two=2)
                for d in range(2):
                    l0 = l00 + d * 128
                    for hh in range(2):
                        psl = slice(hh * 64, hh * 64 + 64)
                        band_block(bpool, qt[pair][psl, l0:l0 + 128], st8, d, hh,
                                   (S - 128) - l0 + EJ, psl, (hh * 64, 0))
                nc.gpsimd.dma_start(
                    out=bass.AP(tensor=ub[pair].tensor,
                                offset=ub[pair].offset + l00 * BPU,
                                ap=[[BPU + 1, 128], [128 * BPU, 2], [1, BW]]),
                    in_=stg)
                # V band rows r, plain: vb[r, j - jbt], jbt = S-128 - r0
                stgv = stgp.tile([128, 2, BW], F16, tag="stgv")
                sv8 = stgv.bitcast(F8).rearrange("p d (c two) -> p d c two", two=2)
                for d in range(2):
                    r0 = l00 + d * 128
                    for hh in range(2):
                        psl = slice(hh * 64, hh * 64 + 64)
                        band_block(bpool, kt[pair][psl, r0:r0 + 128], sv8, d, hh,
                                   (S - 128) - r0, psl, (hh * 64, 0))
                nc.gpsimd.dma_start(
                    out=bass.AP(tensor=vb[pair].tensor,
                                offset=vb[pair].offset + l00 * BP,
                                ap=[[BP, 128], [128 * BP, 2], [1, BW]]),
                    in_=stgv)

            def asm_pair(pair, psr, pvps):
                ebuf = {(hh, lc): [] for hh in range(2) for lc in range(NLC)}
                ready = []

                def do_pv(item):
                    hh, lc, pend, first = item
                    head = 2 * pair + hh
                    # M=128 stationary (65-col stationaries run the PE at half
                    # rate); rows 65:128 of pvt are junk
                    vh = slice(head * 65, head * 65 + 128)
                    lco = lc * LCW
                    pvt = pvps.tile([128, LCW], F32, tag="pv")
                    for k, (rtk, e) in enumerate(pend):
                        for s2 in range(LCW // SCW):
                            s_sl = slice(s2 * SCW, s2 * SCW + SCW)
                            nc.tensor.matmul(pvt[:, s_sl], vhat[:, rtk, vh],
                                             e[:, s_sl], start=(k == 0),
                                             stop=(k == len(pend) - 1))
                    csl = ctxT[head][:, lco:lco + LCW]
                    if first:
                        nc.vector.tensor_copy(out=csl, in_=pvt[0:65, :])
                    else:
                        nc.vector.tensor_add(csl, csl, pvt[0:65, :])

                for rt in range(NLB):
                    r0 = rt * 128
                    # one packed full-row transpose readback per r-tile
                    u_d = udp.tile([128, S], F16, tag="u_d")
                    nc.sync.dma_start(
                        out=u_d,
                        in_=bass.AP(tensor=ub[pair].tensor,
                                    offset=ub[pair].offset + 128 + r0,
                                    ap=[[BPU, S], [1, 128]]),
                        transpose=True)
                    u8 = u_d.bitcast(F8).rearrange("p (c two) -> p c two", two=2)
                    v_d = vdp.tile([128, S], F16, tag="v_d")
                    nc.sync.dma_start(
                        out=v_d,
                        in_=bass.AP(tensor=vb[pair].tensor,
                                    offset=vb[pair].offset + r0 * BP + 127,
                                    ap=[[BP - 1, 128], [1, S]]))
                    v8 = v_d.bitcast(F8).rearrange("p (c two) -> p c two", two=2)
                    # DVE pre-adds the two bias diagonals so the PE injects
                    # them with ONE identity pass instead of two
                    w8 = w8p.tile([128, 2, S], F8, tag="w8")
                    for hh in range(2):
                        nc.vector.tensor_add(w8[:, hh, :], u8[:, :, hh],
                                             v8[:, :, hh])
                    for hh in range(2):
                        psl = slice(hh * 64, hh * 64 + 64)
                        tp = (hh * 64, 0)
                        for lc in range(NLC):
                            lco = lc * LCW
                            sc = psr.tile([128, LCW], F32, tag="ps")
                            for s2 in range(LCW // SCW):
                                s_sl = slice(s2 * SCW, s2 * SCW + SCW)
                                nc.tensor.matmul(
                                    sc[:, s_sl], kt[pair][psl, r0:r0 + 128],
                                    qt[pair][psl, lco + s2 * SCW:lco + s2 * SCW + SCW],
                                    start=True, stop=False, tile_position=tp)
                            for s2 in range(LCW // SCW):
                                s_sl = slice(s2 * SCW, s2 * SCW + SCW)
                                nc.tensor.matmul(
                                    sc[:, s_sl], id8,
                                    w8[:, hh, lco + s2 * SCW:lco + s2 * SCW + SCW],
                                    start=False, stop=True)
                            e_t = expp.tile([128, LCW], F16, tag="e_t")
                            nc.scalar.activation(out=e_t, in_=sc, func=EXP,
                                                 scale=1.0 / np.sqrt(D))
                            ebuf[(hh, lc)].append((rt, e_t))
                    if rt % 2 == 1:
                        # enqueue the closed 2-rt PV window per stream; process
                        # with one-window delay so the trailing exps are never
                        # on the PE's critical path
                        for hh in range(2):
                            for lc in range(NLC):
                                ready.append((hh, lc, ebuf[(hh, lc)], rt == 1))
                                ebuf[(hh, lc)] = []
                        while len(ready) > 4:
                            do_pv(ready.pop(0))
                while ready:
                    do_pv(ready.pop(0))

            # both band phases first (deep dedicated PSUM ring, 8 banks), so
            # the pair-0 xbar readbacks prefetch during the pair-1 band phase
            with tc.tile_pool(name="psrB", bufs=4, space="PSUM") as psrB:
                for pair in range(NPAIR):
                    for g2 in range(0, NLB, 2):
                        band_group(psrB, pair, g2)
            # then both assembly phases: clean PE stream at full clock
            with tc.tile_pool(name="psr", bufs=2, space="PSUM") as psr, \
                 tc.tile_pool(name="pvps", bufs=2, space="PSUM") as pvps:
                for pair in range(NPAIR):
                    asm_pair(pair, psr, pvps)

        # ---------------- Phase 3: transpose ctx, normalize, write out ----------------
        with tc.tile_pool(name="fps", bufs=2, space="PSUM") as fps, \
             tc.tile_pool(name="fsb", bufs=3) as fsb:
            for lt in range(NLB):
                ot = fsb.tile([128, DC], F32, tag="ot")
                for head in range(NH):
                    tp_ps = fps.tile([128, 128], F32, tag="tp_ps")
                    nc.tensor.matmul(tp_ps[:, 0:65], ctxT[head][:, lt * 128:lt * 128 + 128],
                                     id32[0:65, 0:65], is_transpose=True)
                    rec = fsb.tile([128, 1], F32, tag="rec")
                    nc.vector.reciprocal(rec, tp_ps[:, 64:65])
                    nc.vector.tensor_scalar_mul(ot[:, head * 64:head * 64 + 64],
                                                tp_ps[:, 0:64], rec)
                nc.vector.tensor_add(ot, ot, bvb)
                nc.gpsimd.dma_start(out=out[lt * 128:lt * 128 + 128, :], in_=ot)


_PROG = {}


def _get_prog():
    if "p" not in _PROG:
        _PROG["p"] = build_program()
    return _PROG["p"]


def make_in_maps(hidden_states, Wq, bq, Wk, bk, Wv, bv, dist_emb):
    S, HIN = 2048, 1024
    hidden_states = np.asarray(hidden_states, dtype=np.float32)
    Wq = np.asarray(Wq, dtype=np.float16)
    Wk = np.asarray(Wk, dtype=np.float16)
    Wv = np.asarray(Wv, dtype=np.float16)
    bq = np.asarray(bq, dtype=np.float32)
    bk = np.asarray(bk, dtype=np.float32)
    bv = np.asarray(bv, dtype=np.float32)
    dist_emb = np.asarray(dist_emb, dtype=np.float16)

    etp = np.zeros((64, 2 * S), np.float16)
    etp[:, : 2 * S - 1] = dist_emb.T
    # reversed copy for the U band (forward-stride writes): E'[j'] = E[2S-1-j']
    etr = np.zeros((64, 2 * S), np.float16)
    etr[:, 1:] = dist_emb.T[:, ::-1]
    eth = np.concatenate([etp, etr], axis=1)
    et_full = np.ascontiguousarray(np.concatenate([eth, eth], axis=0))

    in_maps = []
    for c in range(N_CORES):
        b = c // 4
        q = c % 4
        cols = slice(q * 256, q * 256 + 256)
        xt = np.ascontiguousarray(hidden_states[b].T.astype(np.float16)).reshape(
            HIN // 128, 128, S)
        in_maps.append({
            "xt": xt,
            "wq": np.ascontiguousarray(Wq[:, cols]).reshape(HIN // 128, 128, 256),
            "wk": np.ascontiguousarray(Wk[:, cols]).reshape(HIN // 128, 128, 256),
            "wv": np.ascontiguousarray(Wv[:, cols]).reshape(HIN // 128, 128, 256),
            "bq": np.ascontiguousarray(bq[cols]),
            "bk": np.ascontiguousarray(bk[cols]),
            "bv": np.ascontiguousarray(bv[cols]),
            "et": et_full,
        })
    return in_maps


def assemble_output(results, B=2, S=2048, HIN=1024):
    full = np.empty((B, S, HIN), np.float32)
    for c in range(N_CORES):
        b = c // 4
        q = c % 4
        full[b, :, q * 256:q * 256 + 256] = results[c]["out"]
    return full


def kernel(hidden_states, Wq, bq, Wk, bk, Wv, bv, dist_emb):
    nc = _get_prog()
    in_maps = make_in_maps(hidden_states, Wq, bq, Wk, bk, Wv, bv, dist_emb)
    res = run_bass_kernel_spmd(nc, in_maps, list(range(N_CORES)))
    return assemble_output(res.results, B=np.asarray(hidden_states).shape[0])


if __name__ == "__main__":
    rng = np.random.default_rng(0)
    B, S, H = 2, 2048, 1024
    inputs = {
        "hidden_states": rng.standard_normal((B, S, H), dtype=np.float32),
        "Wq": rng.standard_normal((H, H), dtype=np.float32) / 32,
        "bq": np.zeros(H, np.float32),
        "Wk": rng.standard_normal((H, H), dtype=np.float32) / 32,
        "bk": np.zeros(H, np.float32),
        "Wv": rng.standard_normal((H, H), dtype=np.float32) / 32,
        "bv": np.zeros(H, np.float32),
        "dist_emb": (rng.standard_normal((2 * 2048 - 1, 64)) * 0.02).astype(np.float32),
    }
    out = kernel(**inputs)
    print("kernel output", out.shape, out.dtype)


# revision 49
# speedup vs baseline: 1.2356x; 1.0453x over previous
"""Trainium2 Bass kernel for BertSelfAttention with relative_key_query position bias.

Full inputs in, full output out; internally sharded over 8 NeuronCores:
data-parallel over batch (B=2) x tensor-parallel over heads (16 heads -> 4 per core).

Per-core algorithm (b fixed, 4 heads, S=2048, d=64), all-fp16 matmul inputs:
  qT,kT = Wq/Wk_slice.T @ x.T   (d-on-partition layout), v natural [S, d]
  scoresT[r,l] = k[r].q[l] + U[l, l-r+2047] + V[r, l-r+2047]
      U = q @ E^T, V = k @ E^T  (E = dist_emb), computed as diagonal BANDS on PE,
      staged to DRAM (sheared for U / plain for V) so the diagonal extraction is
      a plain strided DMA (V) and a 2-byte xbar transpose DMA (U).
  Mega-phase structure: ALL heads' bands first, then all assembly, so the
  diagonal readback DMAs prefetch deeply and the PE never stalls (keeps the
  tensor engine p-state at full clock).
  exp via ScalarE (scale=1/8), unnormalized; softmax sums come free from a
  ones-augmented v in the PV matmul (row 64 of ctxT = sum_r exp).
  V-injection alternates DVE tensor_add / PE identity-matmul to balance engines.
  ctx = (exp^T @ v-hat) / sums, transposed back on PE, written out.
"""

import sys

for _p in ("/opt/trn_rl_repo", "/root/.axon_site/_ro/trn_rl_repo"):
    if _p not in sys.path:
        sys.path.insert(0, _p)

import numpy as np

import concourse.bass as bass
import concourse.tile as tile
from concourse import bacc, mybir
from concourse.bass_utils import run_bass_kernel_spmd
from concourse.masks import make_identity

F32 = mybir.dt.float32
F16 = mybir.dt.float16

N_CORES = 8


def build_program(S=2048, HIN=1024, NH=4, D=64):
    DC = NH * D            # per-core projection output cols
    KC = HIN // 128        # contraction chunks
    NLB = S // 128         # l-blocks / r-tiles
    NLC = 2                # big l-chunks for assembly
    LCW = S // NLC         # l-chunk width
    SH = S - 1             # distance shift (= MAX_POS-1)
    BW = S + 128           # band width per 128-row block (written span)
    BP = BW                # V band row pitch
    BPU = S + 256          # U band row pitch (sheared row span is 2303)
    EJ = 2 * S             # padded E^T cols (E rows 2S-1, pad 1)
    NPAIR = NH // 2

    nc = bacc.Bacc(None, target_bir_lowering=False, debug=False, num_devices=N_CORES)

    xt = nc.declare_dram_parameter("xt", [KC, 128, S], F16, isOutput=False)
    wq = nc.declare_dram_parameter("wq", [KC, 128, DC], F16, isOutput=False)
    wk = nc.declare_dram_parameter("wk", [KC, 128, DC], F16, isOutput=False)
    wv = nc.declare_dram_parameter("wv", [KC, 128, DC], F16, isOutput=False)
    bq = nc.declare_dram_parameter("bq", [DC], F32, isOutput=False)
    bk = nc.declare_dram_parameter("bk", [DC], F32, isOutput=False)
    bv = nc.declare_dram_parameter("bv", [DC], F32, isOutput=False)
    et = nc.declare_dram_parameter("et", [128, 2 * EJ], F16, isOutput=False)
    out = nc.declare_dram_parameter("out", [S, DC], F32, isOutput=True)

    with tile.TileContext(nc) as tc:
        _body(tc, locals())
    nc.compile()
    return nc


def _body(tc, g):
    nc = tc.nc
    S, NH, D, KC, DC = g["S"], g["NH"], g["D"], g["KC"], g["DC"]
    NLB, NLC, LCW, SH, BW, BP, BPU, EJ, NPAIR = (
        g["NLB"], g["NLC"], g["LCW"], g["SH"], g["BW"], g["BP"], g["BPU"],
        g["EJ"], g["NPAIR"])
    xt, wq, wk, wv, bq, bk, bv, et, out = (
        g["xt"], g["wq"], g["wk"], g["wv"], g["bq"], g["bk"], g["bv"], g["et"], g["out"])

    ID = mybir.ActivationFunctionType.Identity
    EXP = mybir.ActivationFunctionType.Exp

    # band chunking: [0,1024) + [1024,2048) psum tiles, [2048,2176) tail tile
    BCHA = [(0, 512), (512, 512)]
    BCHB = [(1024, 512), (1536, 512)]

    from contextlib import ExitStack
    ctx = ExitStack()
    with ctx:
        constp = ctx.enter_context(tc.tile_pool(name="const", bufs=1))
        qkvp = ctx.enter_context(tc.tile_pool(name="qkv", bufs=1))
        ctxp = ctx.enter_context(tc.tile_pool(name="ctxsb", bufs=NH))
        dramp = ctx.enter_context(tc.tile_pool(name="dram", bufs=1, space="DRAM"))

        et_sb = constp.tile([128, 2 * EJ], F16)
        id16 = constp.tile([128, 128], F16)
        make_identity(nc, id16)
        id32 = constp.tile([128, 128], F32)
        make_identity(nc, id32)
        bq_sb = constp.tile([128, NPAIR], F32)
        bk_sb = constp.tile([128, NPAIR], F32)
        for dc in range(NPAIR):
            nc.sync.dma_start(out=bq_sb[:, dc:dc + 1],
                              in_=bass.AP(tensor=bq, offset=dc * 128,
                                          ap=[[1, 128], [0, 1]]))
            nc.sync.dma_start(out=bk_sb[:, dc:dc + 1],
                              in_=bass.AP(tensor=bk, offset=dc * 128,
                                          ap=[[1, 128], [0, 1]]))
        bvb = constp.tile([128, DC], F32)
        nc.gpsimd.dma_start(out=bvb, in_=bass.AP(tensor=bv, offset=0,
                                                 ap=[[0, 128], [1, DC]]))

        qt = [qkvp.tile([128, S], F16, name=f"qt{p}", tag=f"qt{p}") for p in range(NPAIR)]
        kt = [qkvp.tile([128, S], F16, name=f"kt{p}", tag=f"kt{p}") for p in range(NPAIR)]
        # 63 tail cols pad the last head's PV stationary slice to M=128
        vhat = qkvp.tile([128, NLB, NH * 65 + 63], F16)
        ctxT = [ctxp.tile([65, S], F32, name=f"ctxT{h}", tag="ctxT") for h in range(NH)]

        # ---------------- Phase 1: projections ----------------
        with tc.tile_pool(name="xtw", bufs=1) as xtwp, \
             tc.tile_pool(name="pps", bufs=2, space="PSUM") as pps:
            # all startup loads on ONE queue in need-order (concurrent
            # queues halve each other's dispatch rate): weights first, then
            # xt in chunks so the first contraction can start early, then et
            # (only needed by the band phase ~100us in)
            w_sb = {}
            for nm, wt in (("q", wq), ("k", wk), ("v", wv)):
                w_sb[nm] = xtwp.tile([128, KC, DC], F16, name=f"w{nm}", tag=f"w{nm}")
                nc.sync.dma_start(
                    out=w_sb[nm],
                    in_=bass.AP(tensor=wt, offset=0,
                                ap=[[DC, 128], [128 * DC, KC], [1, DC]]))
            xt_sb = xtwp.tile([128, KC, S], F16)
            for k8 in range(KC):
                nc.sync.dma_start(
                    out=xt_sb[:, k8, :],
                    in_=bass.AP(tensor=xt, offset=k8 * 128 * S,
                                ap=[[S, 128], [1, S]]))
            nc.sync.dma_start(out=et_sb, in_=et[:, :])

            PSW = 512
            for dc in range(NPAIR):
                for sc in range(S // PSW):
                    psq = pps.tile([128, PSW], F32, tag="psq")
                    psk = pps.tile([128, PSW], F32, tag="psk")
                    for k8 in range(KC):
                        nc.tensor.matmul(psq, w_sb["q"][:, k8, dc * 128:dc * 128 + 128],
                                         xt_sb[:, k8, sc * PSW:sc * PSW + PSW],
                                         start=(k8 == 0), stop=(k8 == KC - 1))
                    for k8 in range(KC):
                        nc.tensor.matmul(psk, w_sb["k"][:, k8, dc * 128:dc * 128 + 128],
                                         xt_sb[:, k8, sc * PSW:sc * PSW + PSW],
                                         start=(k8 == 0), stop=(k8 == KC - 1))
                    nc.scalar.activation(out=qt[dc][:, sc * PSW:sc * PSW + PSW], in_=psq,
                                         func=ID, bias=bq_sb[:, dc:dc + 1], scale=1.0)
                    nc.scalar.activation(out=kt[dc][:, sc * PSW:sc * PSW + PSW], in_=psk,
                                         func=ID, bias=bk_sb[:, dc:dc + 1], scale=1.0)

            for st in range(NLB):
                psv = pps.tile([128, DC], F32, tag="psv")
                for k8 in range(KC):
                    nc.tensor.matmul(psv, xt_sb[:, k8, st * 128:st * 128 + 128],
                                     w_sb["v"][:, k8, :],
                                     start=(k8 == 0), stop=(k8 == KC - 1))
                nc.vector.tensor_copy(
                    out=vhat[:, st, 0:NH * 65].rearrange("p (h c) -> p h c", h=NH)[:, :, 0:64],
                    in_=psv.rearrange("p (h c) -> p h c", h=NH))
            nc.vector.memset(vhat[:, :, 0:NH * 65].rearrange("p s (h c) -> p s h c", h=NH)[:, :, :, 64:65], 1.0)
            nc.vector.memset(vhat[:, :, NH * 65:], 0.0)

        # per-PAIR bands: each fp16 cell packs (fp8 h_even, fp8 h_odd)
        ub = [dramp.tile([NLB * 128, BPU], F16, name=f"ub{p}", tag=f"ub{p}")
              for p in range(NPAIR)]
        vb = [dramp.tile([NLB * 128, BP], F16, name=f"vb{p}", tag=f"vb{p}")
              for p in range(NPAIR)]

        # ---------------- Phase 2: interleaved bands + assembly ----------------
        # One unified PSUM ring (4 banks) serves both band chunks and score
        # tiles; one [65, S] PV accumulator per head (4 banks).  The SP HWDGE
        # queue carries ONLY the xbar transposes (concurrent transposes on two
        # HWDGE queues corrupt each other through the shared xbar, measured),
        # so it streams them continuously; band writes and v_d reads ride the
        # gpsimd SWDGE queue.  Head h's assembly is interleaved with head
        # h+1's band creation to keep every engine and the xbar busy.
        st_load = {"act": 0.0, "dve": 0.0}

        def stage(dst, src, cols):
            ca = st_load["act"] + cols * 0.833 + 280
            cd = st_load["dve"] + cols * 1.042 + 300
            if ca <= cd:
                st_load["act"] = ca
                nc.scalar.copy(out=dst, in_=src)
            else:
                st_load["dve"] = cd
                nc.vector.tensor_copy(out=dst, in_=src)

        SCW = 512
        F8 = mybir.dt.float8e4
        id8 = constp.tile([128, 128], F8)
        nc.vector.tensor_copy(out=id8, in_=id32)

        with tc.tile_pool(name="stg", bufs=2) as stgp, \
             tc.tile_pool(name="udp", bufs=8) as udp, \
             tc.tile_pool(name="vdp", bufs=3) as vdp, \
             tc.tile_pool(name="w8p", bufs=2) as w8p, \
             tc.tile_pool(name="expp", bufs=21) as expp:

            def band_block(bpool, src, st8, d, hh, jref, psl, tp):
                # one 128-row band block: cols [0, BW) of moving E at jref,
                # staged into fp8 lane hh of the packed fp16 cells
                for (c0, cw) in ((0, 1024), (1024, 1024), (2048, 128)):
                    ps = bpool.tile([128, 1024], F32, tag="ps")
                    for s2 in range(0, cw, SCW):
                        w = min(SCW, cw - s2)
                        nc.tensor.matmul(ps[:, s2:s2 + w], src,
                                         et_sb[psl, jref + c0 + s2:jref + c0 + s2 + w],
                                         start=True, stop=True, tile_position=tp)
                    stage(st8[:, d, c0:c0 + cw, hh], ps[:, 0:cw], cw)

            def band_group(bpool, pair, g2):
                l00 = g2 * 128
                # U band rows l, sheared via reversed E at cols [EJ, 2EJ)
                stg = stgp.tile([128, 2, BW], F16, tag="stg")
                st8 = stg.bitcast(F8).rearrange("p d (c two) -> p d c two", two=2)
                for d in range(2):
                    l0 = l00 + d * 128
                    for hh in range(2):
                        psl = slice(hh * 64, hh * 64 + 64)
                        band_block(bpool, qt[pair][psl, l0:l0 + 128], st8, d, hh,
                                   (S - 128) - l0 + EJ, psl, (hh * 64, 0))
                nc.gpsimd.dma_start(
                    out=bass.AP(tensor=ub[pair].tensor,
                                offset=ub[pair].offset + l00 * BPU,
                                ap=[[BPU + 1, 128], [128 * BPU, 2], [1, BW]]),
                    in_=stg)
                # V band rows r, plain: vb[r, j - jbt], jbt = S-128 - r0
                stgv = stgp.tile([128, 2, BW], F16, tag="stgv")
                sv8 = stgv.bitcast(F8).rearrange("p d (c two) -> p d c two", two=2)
                for d in range(2):
                    r0 = l00 + d * 128
                    for hh in range(2):
                        psl = slice(hh * 64, hh * 64 + 64)
                        band_block(bpool, kt[pair][psl, r0:r0 + 128], sv8, d, hh,
                                   (S - 128) - r0, psl, (hh * 64, 0))
                nc.gpsimd.dma_start(
                    out=bass.AP(tensor=vb[pair].tensor,
                                offset=vb[pair].offset + l00 * BP,
                                ap=[[BP, 128], [128 * BP, 2], [1, BW]]),
                    in_=stgv)

            def asm_pair(pair, psr, pvps):
                ebuf = {(hh, lc): [] for hh in range(2) for lc in range(NLC)}
                ready = []

                def do_pv(item):
                    hh, lc, pend, first = item
                    head = 2 * pair + hh
                    # M=128 stationary (65-col stationaries run the PE at half
                    # rate); rows 65:128 of pvt are junk
                    vh = slice(head * 65, head * 65 + 128)
                    lco = lc * LCW
                    pvt = pvps.tile([128, LCW], F32, tag="pv")
                    for k, (rtk, e) in enumerate(pend):
                        for s2 in range(LCW // SCW):
                            s_sl = slice(s2 * SCW, s2 * SCW + SCW)
                            nc.tensor.matmul(pvt[:, s_sl], vhat[:, rtk, vh],
                                             e[:, s_sl], start=(k == 0),
                                             stop=(k == len(pend) - 1))
                    csl = ctxT[head][:, lco:lco + LCW]
                    if first:
                        nc.vector.tensor_copy(out=csl, in_=pvt[0:65, :])
                    else:
                        nc.vector.tensor_add(csl, csl, pvt[0:65, :])

                for rt in range(NLB):
                    r0 = rt * 128
                    # one packed full-row transpose readback per r-tile
                    u_d = udp.tile([128, S], F16, tag="u_d")
                    nc.sync.dma_start(
                        out=u_d,
                        in_=bass.AP(tensor=ub[pair].tensor,
                                    offset=ub[pair].offset + 128 + r0,
                                    ap=[[BPU, S], [1, 128]]),
                        transpose=True)
                    u8 = u_d.bitcast(F8).rearrange("p (c two) -> p c two", two=2)
                    v_d = vdp.tile([128, S], F16, tag="v_d")
                    nc.sync.dma_start(
                        out=v_d,
                        in_=bass.AP(tensor=vb[pair].tensor,
                                    offset=vb[pair].offset + r0 * BP + 127,
                                    ap=[[BP - 1, 128], [1, S]]))
                    v8 = v_d.bitcast(F8).rearrange("p (c two) -> p c two", two=2)
                    # DVE pre-adds the two bias diagonals so the PE injects
                    # them with ONE identity pass instead of two
                    w8 = w8p.tile([128, 2, S], F8, tag="w8")
                    for hh in range(2):
                        nc.vector.tensor_add(w8[:, hh, :], u8[:, :, hh],
                                             v8[:, :, hh])
                    for hh in range(2):
                        psl = slice(hh * 64, hh * 64 + 64)
                        tp = (hh * 64, 0)
                        for lc in range(NLC):
                            lco = lc * LCW
                            sc = psr.tile([128, LCW], F32, tag="ps")
                            for s2 in range(LCW // SCW):
                                s_sl = slice(s2 * SCW, s2 * SCW + SCW)
                                nc.tensor.matmul(
                                    sc[:, s_sl], kt[pair][psl, r0:r0 + 128],
                                    qt[pair][psl, lco + s2 * SCW:lco + s2 * SCW + SCW],
                                    start=True, stop=False, tile_position=tp)
                            for s2 in range(LCW // SCW):
                                s_sl = slice(s2 * SCW, s2 * SCW + SCW)
                                nc.tensor.matmul(
                                    sc[:, s_sl], id8,
                                    w8[:, hh, lco + s2 * SCW:lco + s2 * SCW + SCW],
                                    start=False, stop=True)
                            e_t = expp.tile([128, LCW], F16, tag="e_t")
                            nc.scalar.activation(out=e_t, in_=sc, func=EXP,
                                                 scale=1.0 / np.sqrt(D))
                            ebuf[(hh, lc)].append((rt, e_t))
                    if rt % 2 == 1:
                        # enqueue the closed 2-rt PV window per stream; process
                        # with one-window delay so the trailing exps are never
                        # on the PE's critical path
                        for hh in range(2):
                            for lc in range(NLC):
                                ready.append((hh, lc, ebuf[(hh, lc)], rt == 1))
                                ebuf[(hh, lc)] = []
                        while len(ready) > 4:
                            do_pv(ready.pop(0))
                while ready:
                    do_pv(ready.pop(0))

            # both band phases first (deep dedicated PSUM ring, 8 banks), so
            # the pair-0 xbar readbacks prefetch during the pair-1 band phase
            with tc.tile_pool(name="psrB", bufs=4, space="PSUM") as psrB:
                for pair in range(NPAIR):
                    for g2 in range(0, NLB, 2):
                        band_group(psrB, pair, g2)
            # then both assembly phases: clean PE stream at full clock
            with tc.tile_pool(name="psr", bufs=2, space="PSUM") as psr, \
                 tc.tile_pool(name="pvps", bufs=2, space="PSUM") as pvps:
                for pair in range(NPAIR):
                    asm_pair(pair, psr, pvps)

        # ---------------- Phase 3: transpose ctx, normalize, write out ----------------
        with tc.tile_pool(name="fps", bufs=2, space="PSUM") as fps, \
             tc.tile_pool(name="fsb", bufs=3) as fsb:
            for lt in range(NLB):
                ot = fsb.tile([128, DC], F32, tag="ot")
                for head in range(NH):
                    tp_ps = fps.tile([128, 128], F32, tag="tp_ps")
                    nc.tensor.matmul(tp_ps[:, 0:65], ctxT[head][:, lt * 128:lt * 128 + 128],
                                     id32[0:65, 0:65], is_transpose=True)
                    rec = fsb.tile([128, 1], F32, tag="rec")
                    nc.vector.reciprocal(rec, tp_ps[:, 64:65])
                    nc.vector.tensor_scalar_mul(ot[:, head * 64:head * 64 + 64],
                                                tp_ps[:, 0:64], rec)
                nc.vector.tensor_add(ot, ot, bvb)
                nc.gpsimd.dma_start(out=out[lt * 128:lt * 128 + 128, :], in_=ot)


_PROG = {}


def _get_prog():
    if "p" not in _PROG:
        _PROG["p"] = build_program()
    return _PROG["p"]


def make_in_maps(hidden_states, Wq, bq, Wk, bk, Wv, bv, dist_emb):
    S, HIN = 2048, 1024
    hidden_states = np.asarray(hidden_states, dtype=np.float32)
    Wq = np.asarray(Wq, dtype=np.float16)
    Wk = np.asarray(Wk, dtype=np.float16)
    Wv = np.asarray(Wv, dtype=np.float16)
    bq = np.asarray(bq, dtype=np.float32)
    bk = np.asarray(bk, dtype=np.float32)
    bv = np.asarray(bv, dtype=np.float32)
    dist_emb = np.asarray(dist_emb, dtype=np.float16)

    etp = np.zeros((64, 2 * S), np.float16)
    etp[:, : 2 * S - 1] = dist_emb.T
    # reversed copy for the U band (forward-stride writes): E'[j'] = E[2S-1-j']
    etr = np.zeros((64, 2 * S), np.float16)
    etr[:, 1:] = dist_emb.T[:, ::-1]
    eth = np.concatenate([etp, etr], axis=1)
    et_full = np.ascontiguousarray(np.concatenate([eth, eth], axis=0))

    in_maps = []
    for c in range(N_CORES):
        b = c // 4
        q = c % 4
        cols = slice(q * 256, q * 256 + 256)
        xt = np.ascontiguousarray(hidden_states[b].T.astype(np.float16)).reshape(
            HIN // 128, 128, S)
        in_maps.append({
            "xt": xt,
            "wq": np.ascontiguousarray(Wq[:, cols]).reshape(HIN // 128, 128, 256),
            "wk": np.ascontiguousarray(Wk[:, cols]).reshape(HIN // 128, 128, 256),
            "wv": np.ascontiguousarray(Wv[:, cols]).reshape(HIN // 128, 128, 256),
            "bq": np.ascontiguousarray(bq[cols]),
            "bk": np.ascontiguousarray(bk[cols]),
            "bv": np.ascontiguousarray(bv[cols]),
            "et": et_full,
        })
    return in_maps


def assemble_output(results, B=2, S=2048, HIN=1024):
    full = np.empty((B, S, HIN), np.float32)
    for c in range(N_CORES):
        b = c // 4
        q = c % 4
        full[b, :, q * 256:q * 256 + 256] = results[c]["out"]
    return full


def kernel(hidden_states, Wq, bq, Wk, bk, Wv, bv, dist_emb):
    nc = _get_prog()
    in_maps = make_in_maps(hidden_states, Wq, bq, Wk, bk, Wv, bv, dist_emb)
    res = run_bass_kernel_spmd(nc, in_maps, list(range(N_CORES)))
    return assemble_output(res.results, B=np.asarray(hidden_states).shape[0])


if __name__ == "__main__":
    rng = np.random.default_rng(0)
    B, S, H = 2, 2048, 1024
    inputs = {
        "hidden_states": rng.standard_normal((B, S, H), dtype=np.float32),
        "Wq": rng.standard_normal((H, H), dtype=np.float32) / 32,
        "bq": np.zeros(H, np.float32),
        "Wk": rng.standard_normal((H, H), dtype=np.float32) / 32,
        "bk": np.zeros(H, np.float32),
        "Wv": rng.standard_normal((H, H), dtype=np.float32) / 32,
        "bv": np.zeros(H, np.float32),
        "dist_emb": (rng.standard_normal((2 * 2048 - 1, 64)) * 0.02).astype(np.float32),
    }
    out = kernel(**inputs)
    print("kernel output", out.shape, out.dtype)
